# revision 1
# baseline (speedup 1.0000x reference)
"""MoE router (linear gate -> softmax -> top-8 indices) on 8 Trainium2 cores.

Strategy (data-parallel over tokens, W replicated):
  - Each core gets 2048 tokens ([2048, 4096] fp32 shard of x).
  - logits^T = W @ x^T computed on the PE: x is transposed on-chip via
    PE transpose-mode (fp32 has no DMA-transpose path), then fp32 matmuls
    with the 128x64 W^T chunks stationary accumulate [64, 512] logit tiles
    in PSUM over the 32 contraction chunks.
  - Softmax is strictly monotonic, so top-k of softmax(logits) == top-k of
    logits; we skip the softmax entirely.
  - Top-8: PE-transpose logits back to [128 tokens, 64 experts], then the
    DVE Max8 / MaxIndex instructions produce the 8 largest values and their
    indices per token (descending, ties -> lowest index first, matching
    jax.lax.top_k).
"""

import numpy as np

import concourse.bass as bass
import concourse.mybir as mybir
import concourse.tile as tile
from concourse import bacc
from concourse.bass_utils import run_bass_kernel_spmd
from concourse.masks import make_identity

N_CORES = 8
N_TOKENS = 16384
D_MODEL = 4096
N_EXPERTS = 64
TOP_K = 8

TPC = N_TOKENS // N_CORES      # tokens per core (2048)
GROUP = 512                    # tokens per matmul group (max fp32 moving dim)
N_GROUPS = TPC // GROUP        # 4
N_SUB = GROUP // 128           # 4 x 128-token blocks per group
N_CHUNK = D_MODEL // 128       # 32 contraction chunks

F32 = mybir.dt.float32
U32 = mybir.dt.uint32

_CACHE: dict = {}


def _build_program():
    nc = bacc.Bacc(
        "TRN2", target_bir_lowering=False, debug=False, num_devices=N_CORES
    )
    x_d = nc.dram_tensor("x", [TPC, D_MODEL], F32, kind="ExternalInput")
    # W^T pre-packed on host to [128, 32*64]: row p, col c*64+e = W[e, c*128+p]
    wt_d = nc.dram_tensor("wt", [128, N_CHUNK * N_EXPERTS], F32, kind="ExternalInput")
    idx_d = nc.dram_tensor("idx", [TPC, TOP_K], U32, kind="ExternalOutput")

    with tile.TileContext(nc) as tc:
        with (
            tc.tile_pool(name="const", bufs=1) as const_pool,
            tc.tile_pool(name="xin", bufs=2 * N_SUB) as x_pool,
            tc.tile_pool(name="xt_ps", bufs=3, space="PSUM") as xt_ps_pool,
            tc.tile_pool(name="xt_sb", bufs=3) as xt_pool,
            tc.tile_pool(name="lg_ps", bufs=2, space="PSUM") as lg_ps_pool,
            tc.tile_pool(name="lg_sb", bufs=2) as lg_pool,
            tc.tile_pool(name="lt_ps", bufs=2, space="PSUM") as lt_ps_pool,
            tc.tile_pool(name="small", bufs=2 * N_SUB) as small_pool,
        ):
            ident = const_pool.tile([128, 128], F32)
            make_identity(nc, ident[:])
            wt_sb = const_pool.tile([128, N_CHUNK, N_EXPERTS], F32)
            nc.sync.dma_start(
                wt_sb[:], wt_d.ap().rearrange("p (c e) -> p c e", c=N_CHUNK)
            )

            for g in range(N_GROUPS):
                x_sb = []
                for b in range(N_SUB):
                    t = x_pool.tile([128, D_MODEL], F32, tag="x")
                    r0 = (g * N_SUB + b) * 128
                    nc.sync.dma_start(t[:], x_d.ap()[r0 : r0 + 128, :])
                    x_sb.append(t)

                lg_ps = lg_ps_pool.tile([N_EXPERTS, GROUP], F32)

                # one-chunk software pipeline: transposes+copy for chunk k+1
                # are emitted before matmul k so the PE never waits on the
                # PSUM->SBUF copy of the chunk it is about to consume
                xt_tiles: list = [None] * N_CHUNK

                def emit_transpose(k):
                    xt_ps = xt_ps_pool.tile([128, GROUP], F32)
                    for b in range(N_SUB):
                        nc.tensor.transpose(
                            xt_ps[:, b * 128 : (b + 1) * 128],
                            x_sb[b][:, k * 128 : (k + 1) * 128],
                            ident[:],
                        )
                    xt_sb = xt_pool.tile([128, GROUP], F32)
                    if k % 2 == 0:
                        nc.vector.tensor_copy(xt_sb[:], xt_ps[:])
                    else:
                        nc.scalar.copy(xt_sb[:], xt_ps[:])
                    xt_tiles[k] = xt_sb

                emit_transpose(0)
                for k in range(N_CHUNK):
                    if k + 1 < N_CHUNK:
                        emit_transpose(k + 1)
                    nc.tensor.matmul(
                        lg_ps[:],
                        wt_sb[:, k],
                        xt_tiles[k][:],
                        start=(k == 0),
                        stop=(k == N_CHUNK - 1),
                    )
                    xt_tiles[k] = None

                lg_sb = lg_pool.tile([N_EXPERTS, GROUP], F32)
                nc.vector.tensor_copy(lg_sb[:], lg_ps[:])

                for b in range(N_SUB):
                    lt_ps = lt_ps_pool.tile([128, N_EXPERTS], F32)
                    nc.tensor.transpose(
                        lt_ps[:],
                        lg_sb[:, b * 128 : (b + 1) * 128],
                        ident[:N_EXPERTS, :N_EXPERTS],
                    )
                    lt_sb = small_pool.tile([128, N_EXPERTS], F32, tag="lt")
                    nc.vector.tensor_copy(lt_sb[:], lt_ps[:])
                    vals = small_pool.tile([128, TOP_K], F32, tag="vals")
                    idxs = small_pool.tile([128, TOP_K], U32, tag="idxs")
                    nc.vector.max(vals[:], lt_sb[:])
                    nc.vector.max_index(idxs[:], vals[:], lt_sb[:])
                    r0 = (g * N_SUB + b) * 128
                    nc.sync.dma_start(idx_d.ap()[r0 : r0 + 128, :], idxs[:])

    nc.compile()
    return nc


def _get_program():
    if "nc" not in _CACHE:
        _CACHE["nc"] = _build_program()
    return _CACHE["nc"]


def _pack_wt(W: np.ndarray) -> np.ndarray:
    # [64, 4096] -> [128, 32*64] with row p, col c*64+e = W[e, c*128+p]
    return np.ascontiguousarray(
        W.astype(np.float32, copy=False)
        .T.reshape(N_CHUNK, 128, N_EXPERTS)
        .transpose(1, 0, 2)
        .reshape(128, N_CHUNK * N_EXPERTS)
    )


def kernel(x: np.ndarray, W: np.ndarray) -> np.ndarray:
    nc = _get_program()
    x = np.ascontiguousarray(x, dtype=np.float32)
    wt = _pack_wt(W)
    in_maps = [
        {"x": x[c * TPC : (c + 1) * TPC], "wt": wt} for c in range(N_CORES)
    ]
    res = run_bass_kernel_spmd(nc, in_maps, core_ids=list(range(N_CORES)))
    out = np.concatenate(
        [res.results[c]["idx"] for c in range(N_CORES)], axis=0
    )
    return out.astype(np.int32)
